# revision 2
# baseline (speedup 1.0000x reference)
import os

import numpy as np

B, CIN, C, H, W, HEADS = 4, 64, 64, 256, 256, 8
EPS = 1e-5
HC = C // HEADS  # 8

# ---------------------------------------------------------------------------
# Device (Trainium via JAX/PJRT) implementation.
#
# Sharding: data-parallel over batch B=4 across the first 4 NeuronCores
# (all conv / attention ops are batch-independent).  BatchNorm batch stats
# are combined on host (per-device partial sums are tiny: 2x64 floats) and
# the normalize+relu epilogue runs as a second small pmap.
# ---------------------------------------------------------------------------

_DEV_STATE = {}


def _setup_jax():
    if "ok" in _DEV_STATE:
        return _DEV_STATE["ok"]
    try:
        import jax
        import jax.numpy as jnp

        # Persistent compilation cache: makes repeat invocations (fresh
        # processes on the same machine) skip neuronx-cc entirely.
        try:
            os.makedirs("/tmp/jax_cc_cache", exist_ok=True)
            jax.config.update("jax_compilation_cache_dir", "/tmp/jax_cc_cache")
            jax.config.update("jax_persistent_cache_min_entry_size_bytes", -1)
            jax.config.update("jax_persistent_cache_min_compile_time_secs", 0.0)
        except Exception:
            pass

        devs = jax.devices()
        if len(devs) < B:
            raise RuntimeError(f"need {B} devices, have {len(devs)}")

        def conv1x1(x, w, b):
            # x: (C,H,W), w: (O,C)
            y = jnp.einsum("oc,chw->ohw", w, x)
            return y + b[:, None, None]

        def dw_h(x, taps, bias):
            # 11-tap depthwise conv along W, zero pad 5.  x: (C,H,W), taps: (C,11)
            xp = jnp.pad(x, ((0, 0), (0, 0), (5, 5)))
            out = bias[:, None, None] * jnp.ones_like(x)
            for j in range(11):
                out = out + taps[:, j, None, None] * xp[:, :, j : j + W]
            return out

        def dw_v(x, taps, bias):
            xp = jnp.pad(x, ((0, 0), (5, 5), (0, 0)))
            out = bias[:, None, None] * jnp.ones_like(x)
            for j in range(11):
                out = out + taps[:, j, None, None] * xp[:, j : j + H, :]
            return out

        def l2n(x):
            n = jnp.sqrt(jnp.sum(x * x, axis=-1, keepdims=True))
            return x / jnp.maximum(n, 1e-12)

        def split_hw(x):  # (C,H,W) -> (HEADS, H, W*HC)
            xr = x.reshape(HEADS, HC, H, W).transpose(0, 2, 3, 1)
            return xr.reshape(HEADS, H, W * HC)

        def split_wh(x):  # (C,H,W) -> (HEADS, W, H*HC)
            xr = x.reshape(HEADS, HC, H, W).transpose(0, 3, 2, 1)
            return xr.reshape(HEADS, W, H * HC)

        def merge_hw(x):  # (HEADS, H, W*HC) -> (C,H,W)
            xr = x.reshape(HEADS, H, W, HC).transpose(0, 3, 1, 2)
            return xr.reshape(C, H, W)

        def merge_wh(x):  # (HEADS, W, H*HC) -> (C,H,W)
            xr = x.reshape(HEADS, W, H, HC).transpose(0, 3, 2, 1)
            return xr.reshape(C, H, W)

        def attend(q, k, v):
            a = jax.nn.softmax(q @ jnp.swapaxes(k, -1, -2), axis=-1)
            return a @ v + q

        def fwd(x_b, p):
            xc = conv1x1(x_b, p["w_in"], p["b_in"])
            var = jnp.var(xc, axis=0, keepdims=True)
            x1 = xc / jnp.sqrt(var + EPS) * p["ln_w"][:, None, None]
            out1 = dw_h(x1, p["taps_h"], p["bias_h"])
            out2 = dw_v(x1, p["taps_v"], p["bias_v"])
            k1 = l2n(split_hw(conv1x1(out1, p["wk1"], p["bk1"])))
            v1 = split_hw(conv1x1(out1, p["wv1"], p["bv1"]))
            k2 = l2n(split_wh(conv1x1(out2, p["wk2"], p["bk2"])))
            v2 = split_wh(conv1x1(out2, p["wv2"], p["bv2"]))
            q1 = conv1x1(out1, p["wq1"], p["bq1"])
            q2 = conv1x1(out2, p["wq2"], p["bq2"])
            self_q1 = l2n(split_hw(q1))
            cross_q1 = l2n(split_wh(q1))
            self_q2 = l2n(split_wh(q2))
            cross_q2 = l2n(split_hw(q2))
            out = (
                merge_hw(attend(self_q1, k1, v1))
                + merge_wh(attend(self_q2, k2, v2))
                + merge_hw(attend(cross_q2, k1, v1))
                + merge_wh(attend(cross_q1, k2, v2))
                + xc
            )
            out = conv1x1(out, p["w_out"], p["b_out"])
            s1 = jnp.sum(out, axis=(1, 2))
            s2 = jnp.sum(out * out, axis=(1, 2))
            return out, s1, s2

        def norm(out_b, scale, bias):
            return jax.nn.relu(out_b * scale[:, None, None] + bias[:, None, None])

        fwd_p = jax.pmap(fwd, in_axes=(0, None), devices=devs[:B])
        norm_p = jax.pmap(norm, in_axes=(0, None, None), devices=devs[:B])

        _DEV_STATE.update(jax=jax, jnp=jnp, fwd_p=fwd_p, norm_p=norm_p, ok=True)
        return True
    except Exception:
        _DEV_STATE["ok"] = False
        return False


def _combine_taps(w3, w7, w11):
    w3 = np.asarray(w3, np.float32).reshape(C, -1)
    w7 = np.asarray(w7, np.float32).reshape(C, -1)
    w11 = np.asarray(w11, np.float32).reshape(C, -1)
    comb = w11.copy()
    comb[:, 2:9] += w7
    comb[:, 4:7] += w3
    return comb


def _kernel_device(x, params):
    jnp = _DEV_STATE["jnp"]
    fwd_p = _DEV_STATE["fwd_p"]
    norm_p = _DEV_STATE["norm_p"]

    out_pre, s1, s2 = fwd_p(jnp.asarray(x), params)
    s1 = np.asarray(s1).sum(axis=0)
    s2 = np.asarray(s2).sum(axis=0)
    n = B * H * W
    mu = s1 / n
    var = s2 / n - mu * mu
    inv = 1.0 / np.sqrt(var + EPS)
    scale = (params["bn_g"] * inv).astype(np.float32)
    bias = (params["bn_b"] - mu * scale).astype(np.float32)
    out = norm_p(out_pre, jnp.asarray(scale), jnp.asarray(bias))
    return np.asarray(out, dtype=np.float32)


# ---------------------------------------------------------------------------
# NumPy fallback (reference-equivalent), used only if the device path fails.
# ---------------------------------------------------------------------------


def _np_conv1x1(x, w, b):
    y = np.einsum("oc,bchw->bohw", w, x, optimize=True)
    return y + b[None, :, None, None]


def _np_dw_h(x, taps, bias):
    xp = np.pad(x, ((0, 0), (0, 0), (0, 0), (5, 5)))
    out = np.zeros_like(x)
    for j in range(11):
        out += taps[None, :, j, None, None] * xp[:, :, :, j : j + W]
    return out + bias[None, :, None, None]


def _np_dw_v(x, taps, bias):
    xp = np.pad(x, ((0, 0), (0, 0), (5, 5), (0, 0)))
    out = np.zeros_like(x)
    for j in range(11):
        out += taps[None, :, j, None, None] * xp[:, :, j : j + H, :]
    return out + bias[None, :, None, None]


def _np_l2n(x):
    n = np.sqrt(np.sum(x * x, axis=-1, keepdims=True))
    return x / np.maximum(n, 1e-12)


def _np_split_hw(x):
    b, ch, h, w = x.shape
    c = ch // HEADS
    return x.reshape(b, HEADS, c, h, w).transpose(0, 1, 3, 4, 2).reshape(b, HEADS, h, w * c)


def _np_split_wh(x):
    b, ch, h, w = x.shape
    c = ch // HEADS
    return x.reshape(b, HEADS, c, h, w).transpose(0, 1, 4, 3, 2).reshape(b, HEADS, w, h * c)


def _np_merge_hw(x, h, w):
    b, hd, _, wc = x.shape
    c = wc // w
    return x.reshape(b, hd, h, w, c).transpose(0, 1, 4, 2, 3).reshape(b, hd * c, h, w)


def _np_merge_wh(x, h, w):
    b, hd, _, hc = x.shape
    c = hc // h
    return x.reshape(b, hd, w, h, c).transpose(0, 1, 4, 3, 2).reshape(b, hd * c, h, w)


def _np_attend(q, k, v):
    logits = np.matmul(q, np.swapaxes(k, -1, -2))
    logits -= logits.max(axis=-1, keepdims=True)
    e = np.exp(logits)
    a = e / e.sum(axis=-1, keepdims=True)
    return np.matmul(a, v) + q


def _kernel_numpy(x, w_in, b_in, ln_w, taps_h, bias_h, taps_v, bias_v, wq1, bq1,
                  wq2, bq2, wk1, bk1, wk2, bk2, wv1, bv1, wv2, bv2, w_out, b_out,
                  bn_g, bn_b):
    h, w = x.shape[-2:]
    xc = _np_conv1x1(x, w_in, b_in)
    var = xc.var(axis=1, keepdims=True)
    x1 = xc / np.sqrt(var + EPS) * ln_w[None, :, None, None]
    out1 = _np_dw_h(x1, taps_h, bias_h)
    out2 = _np_dw_v(x1, taps_v, bias_v)
    k1 = _np_l2n(_np_split_hw(_np_conv1x1(out1, wk1, bk1)))
    v1 = _np_split_hw(_np_conv1x1(out1, wv1, bv1))
    k2 = _np_l2n(_np_split_wh(_np_conv1x1(out2, wk2, bk2)))
    v2 = _np_split_wh(_np_conv1x1(out2, wv2, bv2))
    q1 = _np_conv1x1(out1, wq1, bq1)
    q2 = _np_conv1x1(out2, wq2, bq2)
    out = (_np_merge_hw(_np_attend(_np_l2n(_np_split_hw(q1)), k1, v1), h, w)
           + _np_merge_wh(_np_attend(_np_l2n(_np_split_wh(q2)), k2, v2), h, w)
           + _np_merge_hw(_np_attend(_np_l2n(_np_split_hw(q2)), k1, v1), h, w)
           + _np_merge_wh(_np_attend(_np_l2n(_np_split_wh(q1)), k2, v2), h, w)
           + xc)
    out = _np_conv1x1(out, w_out, b_out)
    mu = out.mean(axis=(0, 2, 3), keepdims=True)
    var = out.var(axis=(0, 2, 3), keepdims=True)
    out = (out - mu) / np.sqrt(var + EPS) * bn_g[None, :, None, None] \
        + bn_b[None, :, None, None]
    return np.maximum(out, 0.0).astype(np.float32)


def kernel(x, w_in, b_in, ln_w, dw01_w, dw01_b, dw02_w, dw02_b, dw11_w, dw11_b,
           dw12_w, dw12_b, dw21_w, dw21_b, dw22_w, dw22_b, wq1, bq1, wq2, bq2,
           wk1, bk1, wk2, bk2, wv1, bv1, wv2, bv2, w_out, b_out, bn_g, bn_b):
    x = np.asarray(x, dtype=np.float32)
    f32 = lambda a: np.asarray(a, dtype=np.float32)

    taps_h = _combine_taps(dw01_w, dw11_w, dw21_w)
    bias_h = (f32(dw01_b) + f32(dw11_b) + f32(dw21_b)).astype(np.float32)
    taps_v = _combine_taps(dw02_w, dw12_w, dw22_w)
    bias_v = (f32(dw02_b) + f32(dw12_b) + f32(dw22_b)).astype(np.float32)

    params = dict(
        w_in=f32(w_in), b_in=f32(b_in), ln_w=f32(ln_w),
        taps_h=taps_h, bias_h=bias_h, taps_v=taps_v, bias_v=bias_v,
        wq1=f32(wq1), bq1=f32(bq1), wq2=f32(wq2), bq2=f32(bq2),
        wk1=f32(wk1), bk1=f32(bk1), wk2=f32(wk2), bk2=f32(bk2),
        wv1=f32(wv1), bv1=f32(bv1), wv2=f32(wv2), bv2=f32(bv2),
        w_out=f32(w_out), b_out=f32(b_out), bn_g=f32(bn_g), bn_b=f32(bn_b),
    )

    if _setup_jax():
        try:
            return _kernel_device(x, params)
        except Exception:
            pass

    return _kernel_numpy(
        x, params["w_in"], params["b_in"], params["ln_w"], taps_h, bias_h,
        taps_v, bias_v, params["wq1"], params["bq1"], params["wq2"], params["bq2"],
        params["wk1"], params["bk1"], params["wk2"], params["bk2"],
        params["wv1"], params["bv1"], params["wv2"], params["bv2"],
        params["w_out"], params["b_out"], params["bn_g"], params["bn_b"])


# revision 5
# speedup vs baseline: 8.6902x; 8.6902x over previous
import os

import numpy as np

B, CIN, C, H, W, HEADS = 4, 64, 64, 256, 256, 8
EPS = 1e-5
HC = C // HEADS  # 8

# ---------------------------------------------------------------------------
# Device (Trainium via JAX/PJRT) implementation.
#
# Sharding: data-parallel over batch B=4 across the first 4 NeuronCores
# (all conv / attention ops are batch-independent).  BatchNorm batch stats
# are combined on host (per-device partial sums are tiny: 2x64 floats) and
# the normalize+relu epilogue runs as a second small pmap.
# ---------------------------------------------------------------------------

_DEV_STATE = {}


def _setup_jax():
    if "ok" in _DEV_STATE:
        return _DEV_STATE["ok"]
    try:
        import jax
        import jax.numpy as jnp

        # Persistent compilation cache: makes repeat invocations (fresh
        # processes on the same machine) skip neuronx-cc entirely.
        try:
            os.makedirs("/tmp/jax_cc_cache", exist_ok=True)
            jax.config.update("jax_compilation_cache_dir", "/tmp/jax_cc_cache")
            jax.config.update("jax_persistent_cache_min_entry_size_bytes", -1)
            jax.config.update("jax_persistent_cache_min_compile_time_secs", 0.0)
        except Exception:
            pass

        devs = jax.devices()
        if len(devs) < B:
            raise RuntimeError(f"need {B} devices, have {len(devs)}")

        def conv1x1(x, w, b):
            # x: (C,H,W), w: (O,C)
            y = jnp.einsum("oc,chw->ohw", w, x)
            return y + b[:, None, None]

        def dwconv(x, w, b, ph, pw):
            # depthwise strip conv; x: (C,H,W), w: (C,1,kh,kw)
            y = jax.lax.conv_general_dilated(
                x[None], w, (1, 1), [(ph, ph), (pw, pw)],
                feature_group_count=C,
                dimension_numbers=("NCHW", "OIHW", "NCHW"))[0]
            return y + b[:, None, None]

        def l2n(x):
            n = jnp.sqrt(jnp.sum(x * x, axis=-1, keepdims=True))
            return x / jnp.maximum(n, 1e-12)

        def split_hw(x):  # (C,H,W) -> (HEADS, H, W*HC)
            xr = x.reshape(HEADS, HC, H, W).transpose(0, 2, 3, 1)
            return xr.reshape(HEADS, H, W * HC)

        def split_wh(x):  # (C,H,W) -> (HEADS, W, H*HC)
            xr = x.reshape(HEADS, HC, H, W).transpose(0, 3, 2, 1)
            return xr.reshape(HEADS, W, H * HC)

        def merge_hw(x):  # (HEADS, H, W*HC) -> (C,H,W)
            xr = x.reshape(HEADS, H, W, HC).transpose(0, 3, 1, 2)
            return xr.reshape(C, H, W)

        def merge_wh(x):  # (HEADS, W, H*HC) -> (C,H,W)
            xr = x.reshape(HEADS, W, H, HC).transpose(0, 3, 2, 1)
            return xr.reshape(C, H, W)

        def attend(q, k, v):
            a = jax.nn.softmax(q @ jnp.swapaxes(k, -1, -2), axis=-1)
            return a @ v + q

        def fwd(x_b, p):
            xc = conv1x1(x_b, p["w_in"], p["b_in"])
            var = jnp.var(xc, axis=0, keepdims=True)
            x1 = xc / jnp.sqrt(var + EPS) * p["ln_w"][:, None, None]
            out1 = dwconv(x1, p["taps_h4"], p["bias_h"], 0, 5)
            out2 = dwconv(x1, p["taps_v4"], p["bias_v"], 5, 0)
            k1 = l2n(split_hw(conv1x1(out1, p["wk1"], p["bk1"])))
            v1 = split_hw(conv1x1(out1, p["wv1"], p["bv1"]))
            k2 = l2n(split_wh(conv1x1(out2, p["wk2"], p["bk2"])))
            v2 = split_wh(conv1x1(out2, p["wv2"], p["bv2"]))
            q1 = conv1x1(out1, p["wq1"], p["bq1"])
            q2 = conv1x1(out2, p["wq2"], p["bq2"])
            self_q1 = l2n(split_hw(q1))
            cross_q1 = l2n(split_wh(q1))
            self_q2 = l2n(split_wh(q2))
            cross_q2 = l2n(split_hw(q2))
            out = (
                merge_hw(attend(self_q1, k1, v1))
                + merge_wh(attend(self_q2, k2, v2))
                + merge_hw(attend(cross_q2, k1, v1))
                + merge_wh(attend(cross_q1, k2, v2))
                + xc
            )
            out = conv1x1(out, p["w_out"], p["b_out"])
            s1 = jnp.sum(out, axis=(1, 2))
            s2 = jnp.sum(out * out, axis=(1, 2))
            return out, s1, s2

        def norm(out_b, scale, bias):
            return jax.nn.relu(out_b * scale[:, None, None] + bias[:, None, None])

        fwd_p = jax.pmap(fwd, in_axes=(0, None), devices=devs[:B])
        norm_p = jax.pmap(norm, in_axes=(0, None, None), devices=devs[:B])

        _DEV_STATE.update(jax=jax, jnp=jnp, fwd_p=fwd_p, norm_p=norm_p, ok=True)
        return True
    except Exception:
        _DEV_STATE["ok"] = False
        return False


def _combine_taps(w3, w7, w11):
    w3 = np.asarray(w3, np.float32).reshape(C, -1)
    w7 = np.asarray(w7, np.float32).reshape(C, -1)
    w11 = np.asarray(w11, np.float32).reshape(C, -1)
    comb = w11.copy()
    comb[:, 2:9] += w7
    comb[:, 4:7] += w3
    return comb


def _kernel_device(x, params):
    jnp = _DEV_STATE["jnp"]
    fwd_p = _DEV_STATE["fwd_p"]
    norm_p = _DEV_STATE["norm_p"]

    out_pre, s1, s2 = fwd_p(jnp.asarray(x), params)
    s1 = np.asarray(s1).sum(axis=0)
    s2 = np.asarray(s2).sum(axis=0)
    n = B * H * W
    mu = s1 / n
    var = s2 / n - mu * mu
    inv = 1.0 / np.sqrt(var + EPS)
    scale = (params["bn_g"] * inv).astype(np.float32)
    bias = (params["bn_b"] - mu * scale).astype(np.float32)
    out = norm_p(out_pre, jnp.asarray(scale), jnp.asarray(bias))
    return np.asarray(out, dtype=np.float32)


# ---------------------------------------------------------------------------
# NumPy fallback (reference-equivalent), used only if the device path fails.
# ---------------------------------------------------------------------------


def _np_conv1x1(x, w, b):
    y = np.einsum("oc,bchw->bohw", w, x, optimize=True)
    return y + b[None, :, None, None]


def _np_dw_h(x, taps, bias):
    xp = np.pad(x, ((0, 0), (0, 0), (0, 0), (5, 5)))
    out = np.zeros_like(x)
    for j in range(11):
        out += taps[None, :, j, None, None] * xp[:, :, :, j : j + W]
    return out + bias[None, :, None, None]


def _np_dw_v(x, taps, bias):
    xp = np.pad(x, ((0, 0), (0, 0), (5, 5), (0, 0)))
    out = np.zeros_like(x)
    for j in range(11):
        out += taps[None, :, j, None, None] * xp[:, :, j : j + H, :]
    return out + bias[None, :, None, None]


def _np_l2n(x):
    n = np.sqrt(np.sum(x * x, axis=-1, keepdims=True))
    return x / np.maximum(n, 1e-12)


def _np_split_hw(x):
    b, ch, h, w = x.shape
    c = ch // HEADS
    return x.reshape(b, HEADS, c, h, w).transpose(0, 1, 3, 4, 2).reshape(b, HEADS, h, w * c)


def _np_split_wh(x):
    b, ch, h, w = x.shape
    c = ch // HEADS
    return x.reshape(b, HEADS, c, h, w).transpose(0, 1, 4, 3, 2).reshape(b, HEADS, w, h * c)


def _np_merge_hw(x, h, w):
    b, hd, _, wc = x.shape
    c = wc // w
    return x.reshape(b, hd, h, w, c).transpose(0, 1, 4, 2, 3).reshape(b, hd * c, h, w)


def _np_merge_wh(x, h, w):
    b, hd, _, hc = x.shape
    c = hc // h
    return x.reshape(b, hd, w, h, c).transpose(0, 1, 4, 3, 2).reshape(b, hd * c, h, w)


def _np_attend(q, k, v):
    logits = np.matmul(q, np.swapaxes(k, -1, -2))
    logits -= logits.max(axis=-1, keepdims=True)
    e = np.exp(logits)
    a = e / e.sum(axis=-1, keepdims=True)
    return np.matmul(a, v) + q


def _kernel_numpy(x, w_in, b_in, ln_w, taps_h, bias_h, taps_v, bias_v, wq1, bq1,
                  wq2, bq2, wk1, bk1, wk2, bk2, wv1, bv1, wv2, bv2, w_out, b_out,
                  bn_g, bn_b):
    h, w = x.shape[-2:]
    xc = _np_conv1x1(x, w_in, b_in)
    var = xc.var(axis=1, keepdims=True)
    x1 = xc / np.sqrt(var + EPS) * ln_w[None, :, None, None]
    out1 = _np_dw_h(x1, taps_h, bias_h)
    out2 = _np_dw_v(x1, taps_v, bias_v)
    k1 = _np_l2n(_np_split_hw(_np_conv1x1(out1, wk1, bk1)))
    v1 = _np_split_hw(_np_conv1x1(out1, wv1, bv1))
    k2 = _np_l2n(_np_split_wh(_np_conv1x1(out2, wk2, bk2)))
    v2 = _np_split_wh(_np_conv1x1(out2, wv2, bv2))
    q1 = _np_conv1x1(out1, wq1, bq1)
    q2 = _np_conv1x1(out2, wq2, bq2)
    out = (_np_merge_hw(_np_attend(_np_l2n(_np_split_hw(q1)), k1, v1), h, w)
           + _np_merge_wh(_np_attend(_np_l2n(_np_split_wh(q2)), k2, v2), h, w)
           + _np_merge_hw(_np_attend(_np_l2n(_np_split_hw(q2)), k1, v1), h, w)
           + _np_merge_wh(_np_attend(_np_l2n(_np_split_wh(q1)), k2, v2), h, w)
           + xc)
    out = _np_conv1x1(out, w_out, b_out)
    mu = out.mean(axis=(0, 2, 3), keepdims=True)
    var = out.var(axis=(0, 2, 3), keepdims=True)
    out = (out - mu) / np.sqrt(var + EPS) * bn_g[None, :, None, None] \
        + bn_b[None, :, None, None]
    return np.maximum(out, 0.0).astype(np.float32)


def kernel(x, w_in, b_in, ln_w, dw01_w, dw01_b, dw02_w, dw02_b, dw11_w, dw11_b,
           dw12_w, dw12_b, dw21_w, dw21_b, dw22_w, dw22_b, wq1, bq1, wq2, bq2,
           wk1, bk1, wk2, bk2, wv1, bv1, wv2, bv2, w_out, b_out, bn_g, bn_b):
    x = np.asarray(x, dtype=np.float32)
    f32 = lambda a: np.asarray(a, dtype=np.float32)

    taps_h = _combine_taps(dw01_w, dw11_w, dw21_w)
    bias_h = (f32(dw01_b) + f32(dw11_b) + f32(dw21_b)).astype(np.float32)
    taps_v = _combine_taps(dw02_w, dw12_w, dw22_w)
    bias_v = (f32(dw02_b) + f32(dw12_b) + f32(dw22_b)).astype(np.float32)

    params = dict(
        w_in=f32(w_in), b_in=f32(b_in), ln_w=f32(ln_w),
        taps_h4=taps_h.reshape(C, 1, 1, 11), bias_h=bias_h,
        taps_v4=taps_v.reshape(C, 1, 11, 1), bias_v=bias_v,
        wq1=f32(wq1), bq1=f32(bq1), wq2=f32(wq2), bq2=f32(bq2),
        wk1=f32(wk1), bk1=f32(bk1), wk2=f32(wk2), bk2=f32(bk2),
        wv1=f32(wv1), bv1=f32(bv1), wv2=f32(wv2), bv2=f32(bv2),
        w_out=f32(w_out), b_out=f32(b_out), bn_g=f32(bn_g), bn_b=f32(bn_b),
    )

    if _setup_jax():
        try:
            return _kernel_device(x, params)
        except Exception:
            pass

    return _kernel_numpy(
        x, params["w_in"], params["b_in"], params["ln_w"], taps_h, bias_h,
        taps_v, bias_v, params["wq1"], params["bq1"], params["wq2"], params["bq2"],
        params["wk1"], params["bk1"], params["wk2"], params["bk2"],
        params["wv1"], params["bv1"], params["wv2"], params["bv2"],
        params["w_out"], params["b_out"], params["bn_g"], params["bn_b"])
